# revision 1
# baseline (speedup 1.0000x reference)
"""Trainium2 Bass kernel for nn_MmdLoss (RBF-MMD + area loss).

Contract: kernel(**inputs) takes FULL [8, 262144] f32 inputs, returns FULL
[8] f32 output. Internally: data-parallel over batch across 8 NeuronCores
(sample b on core b); one tiny AllGather provides the batch-global sums that
define the stochastic selection thresholds.

Exact math reformulations of the reference (see reference.py):
  - Image is 512x512, pooled 4x4 -> 128x128 grid (N = 16384).
  - The [N,N] RBF kernel is separable: K = K1 (x) K1 (Kronecker) with
    K1[a,b] = exp(-(a-b)^2/128), symmetric 128x128. Hence for grid-shaped
    Qm, Pm [128,128]:  q^T K p = sum(Qm * (K1 @ Pm @ K1)).
  - avg-pool + per-sample normalization == sum-pool + normalization.
  - maxpool4x4(sel) == (maxpool4x4(ln x - ln u) > ln th): the selection
    x > u*th is equivalent to ln x - ln u > ln th (th >= 0.01 > 0), and the
    max-pool commutes with the threshold compare -- so ALL per-pixel work is
    threshold-independent and overlaps the collective.
    Edge cases: x=0 -> -inf (never selected, matches x>0 test);
    u=0 -> +inf (always selected, matches x>0); both zero -> NaN -> not
    selected (reference: 0 > 0 false). All consistent.
  - position = 0.5*(a^2*Sqq + b^2*Spp - 2ab*Sqp), a = 1/sum(Qraw),
    b = 1/sum(Praw), Sxy = sum(Xm * (K1 @ Ym @ K1)) on raw (unnormalized)
    sum-pooled masked weights.
  - area = ((Sx - St)/16)^2 / 262144 with Sx,St per-sample full-image sums.
  - th_x = max(Sx_tot/4000, 0.01), th_t = max(St_tot/800, 0.01) where
    *_tot are batch-global sums (AllGather of per-sample sums + local
    8-element reduce; AG has a ~2x lower latency floor than AllReduce).

Layout per core: each [262144] sample is viewed as [128, 2048]; partition i
holds image rows 4i..4i+3, so a 4x4 pool is a reduce over the free-dim view
(j, k, c) -> j with f = k*512 + j*4 + c  (k = row-in-group, j = pooled col,
c = col-in-group).

Engine split: ACT computes per-sample sums (copy+accum), the four Ln
transforms, and exp(maxpool); DVE does the pooled reduces, log-differences,
selection, and the final scalar chain; PE does the tiny matmuls (partition
reductions, threshold broadcast, and the K1-sandwich products). All
threshold-independent work overlaps the ~45us collective window; the
post-collective tail is ~10us.

Build workarounds for this container's walrus (see _patch_tile_drain and the
absorber matmuls): per-instruction sync-wait slots are tiny (Matmult=1), so
the Tile tail drain is split per-semaphore and PE pre-observes DVE/DMA sems.
"""

import numpy as np

B = 8
L = 262144
M = 128          # pooled grid side
NCORES = 8
SIGMA2 = 64.0

_CACHE = {}


def _patch_tile_drain():
    """This container's walrus rejects the Tile kernel-tail drain: it carries
    one sync wait per live semaphore (13 here) on a single SP CTRL
    instruction, which overflows the struct's wait slots ("Too many sync
    wait commands"). Split it into one drain per semaphore instead."""
    import concourse.tile as tile
    from concourse.tile_scheduler import N_PROCS
    from concourse.vector_clock import ScopedClock, VectorClock

    if getattr(tile.TileContext, "_ant_split_drain", False):
        return

    def _drain_and_barrier(self, tick_clock, wait_clock):
        nc = self.nc
        gc = tick_clock.global_clock
        for p in range(N_PROCS):
            if gc[p] > 0:
                vals = [0] * N_PROCS
                vals[p] = gc[p]
                d = nc.sync.drain()
                wait_clock.add_sem_waits(
                    d.ins, ScopedClock({None: VectorClock(vals)})
                )
        nc.all_engine_barrier()
        assert self.sems is not None
        popped = nc._tile_sem_poison_stack.pop()
        assert popped is self._sem_poison
        nc.clear_and_free_semaphores(list(self.sems.allocated().values()))
        nc.all_engine_barrier()

    tile.TileContext._drain_and_barrier = _drain_and_barrier
    tile.TileContext._ant_split_drain = True


def _patch_sim_credit_remote_sem(sem):
    """Single-core CoreSims (Tile scheduling pass, trace validation) can never
    model peer-driven remote-sem increments, so a raw wait on one deadlocks
    them. Credit the sem up-front in any sim without a MultiCoreSim parent;
    hardware semantics are unchanged."""
    import concourse.bass_interp as bass_interp
    from concourse.bass import create_sync_update

    if not hasattr(bass_interp.CoreSim, "_ant_orig_event_loop"):
        bass_interp.CoreSim._ant_orig_event_loop = bass_interp.CoreSim.event_loop

        def event_loop(self):
            for s in getattr(bass_interp.CoreSim, "_ant_credit_sems", ()):
                if self.parent is None:
                    try:
                        self.update_semaphore(create_sync_update(s, 16))
                    except Exception:
                        pass
            return bass_interp.CoreSim._ant_orig_event_loop(self)

        bass_interp.CoreSim.event_loop = event_loop
    sems = list(getattr(bass_interp.CoreSim, "_ant_credit_sems", ()))
    sems.append(sem)
    bass_interp.CoreSim._ant_credit_sems = sems


def _build_bass():
    import concourse.bass as bass
    import concourse.mybir as mybir
    import concourse.tile as tile

    _patch_tile_drain()

    fp32 = mybir.dt.float32
    Alu = mybir.AluOpType
    AX = mybir.AxisListType
    AF = mybir.ActivationFunctionType

    import os

    debug = bool(os.environ.get("MMD_KERNEL_DEBUG"))
    use_collective = not bool(os.environ.get("MMD_USE_RDMA"))
    debug2 = bool(os.environ.get("MMD_KERNEL_DEBUG2"))

    nc = bass.Bass(trn_type="TRN2", num_devices=NCORES)

    x_d = nc.dram_tensor("x", [128, 2048], fp32, kind="ExternalInput")
    t_d = nc.dram_tensor("t", [128, 2048], fp32, kind="ExternalInput")
    ux_d = nc.dram_tensor("ux", [128, 2048], fp32, kind="ExternalInput")
    ut_d = nc.dram_tensor("ut", [128, 2048], fp32, kind="ExternalInput")
    out_d = nc.dram_tensor("out", [1, 1], fp32, kind="ExternalOutput")

    # K1 separable RBF factor, embedded in the NEFF as a constant.
    r = np.arange(M, dtype=np.float64)
    k1_np = np.exp(-((r[:, None] - r[None, :]) ** 2) / (2.0 * SIGMA2)).astype(
        np.float32
    )
    k1_d = nc.inline_tensor(k1_np, name="k1c")

    def pool_view(ap):
        return ap.rearrange("p (k j c) -> p j k c", k=4, j=128, c=4)

    with tile.TileContext(nc) as tc:
        with (
            tc.tile_pool(name="big", bufs=1) as big,
            tc.tile_pool(name="small", bufs=1) as small,
            tc.tile_pool(name="psum", bufs=1, space="PSUM") as psum,
            tc.tile_pool(name="dram", bufs=1, space="DRAM") as dram,
        ):
            # ---- input DMAs (k1 tiny + first; x,t gate the collective) -----
            k1_s = small.tile([128, 128], fp32, name="k1_s")
            nc.sync.dma_start(k1_s[:, :], k1_d[:, :])

            x_s = big.tile([128, 2048], fp32, name="x_s")
            t_s = big.tile([128, 2048], fp32, name="t_s")
            ux_s = big.tile([128, 2048], fp32, name="ux_s")
            ut_s = big.tile([128, 2048], fp32, name="ut_s")
            nc.sync.dma_start(x_s[:, :], x_d[:, :])
            nc.sync.dma_start(t_s[:, :], t_d[:, :])
            nc.sync.dma_start(ut_s[:, :], ut_d[:, :])
            nc.sync.dma_start(ux_s[:, :], ux_d[:, :])

            ones_p = small.tile([128, 1], fp32, name="ones_p")
            nc.vector.memset(ones_p[:, :], 1.0)
            ones_f = small.tile([8, 128], fp32, name="ones_f")
            nc.vector.memset(ones_f[:, :], 1.0)

            # ---- ACT: per-sample sums first (gate the collective), then Ln -
            junk = big.tile([128, 2048], fp32, name="junk")
            ss = small.tile([128, 2], fp32, name="ss")
            nc.scalar.activation(junk[:, :], x_s[:, :], AF.Copy, accum_out=ss[:, 0:1])
            nc.scalar.activation(junk[:, :], t_s[:, :], AF.Copy, accum_out=ss[:, 1:2])

            lx = big.tile([128, 2048], fp32, name="lx")
            lt = big.tile([128, 2048], fp32, name="lt")
            lux = big.tile([128, 2048], fp32, name="lux")
            lut = big.tile([128, 2048], fp32, name="lut")
            nc.scalar.activation(lt[:, :], t_s[:, :], AF.Ln)
            nc.scalar.activation(lut[:, :], ut_s[:, :], AF.Ln)
            nc.scalar.activation(lx[:, :], x_s[:, :], AF.Ln)
            nc.scalar.activation(lux[:, :], ux_s[:, :], AF.Ln)

            # PE instructions can carry only ONE cross-engine sync wait
            # (walrus S3_LW slot limit). Each engine's semaphore is
            # monotonic, so these two absorber matmuls make PE observe the
            # DVE memsets and the k1 DMA once; every later matmul then needs
            # at most one new wait.
            dum_p = psum.tile([128, 1], fp32, name="dum_p")
            aq_p = psum.tile([128, 128], fp32, name="aq_p")
            nc.tensor.matmul(
                dum_p[:, :], lhsT=ones_f[:, :], rhs=ones_f[0:8, 0:1],
                start=True, stop=True,
            )
            nc.tensor.matmul(
                aq_p[:, 0:1], lhsT=k1_s[:, :], rhs=k1_s[:, 0:1],
                start=True, stop=True,
            )

            # ---- pooled sums (DVE) -> per-sample sums -> AllGather ---------
            xa = small.tile([128, 128], fp32, name="xa")
            ta = small.tile([128, 128], fp32, name="ta")
            nc.vector.tensor_reduce(
                out=xa[:, :], in_=pool_view(x_s[:, :]), axis=AX.XY, op=Alu.add
            )
            nc.vector.tensor_reduce(
                out=ta[:, :], in_=pool_view(t_s[:, :]), axis=AX.XY, op=Alu.add
            )
            ssamp_p = psum.tile([1, 2], fp32, name="ssamp_p")
            nc.tensor.matmul(
                ssamp_p[:, :], lhsT=ones_p[:, :], rhs=ss[:, :], start=True, stop=True
            )
            ssamp = small.tile([1, 2], fp32, name="ssamp")
            nc.vector.tensor_copy(ssamp[:, :], ssamp_p[:, :])

            ag_sb = small.tile([8, 2], fp32, name="ag_sb")
            if use_collective:
                cc_in = dram.tile([1, 2], fp32, name="cc_in")
                cc_out = dram.tile([8, 2], fp32, name="cc_out")
                nc.sync.dma_start(cc_in[:, :], ssamp[:, :])
                nc.gpsimd.collective_compute(
                    "AllGather",
                    Alu.bypass,
                    replica_groups=[list(range(NCORES))],
                    ins=[cc_in[:, :]],
                    outs=[cc_out[:, :]],
                )
                nc.sync.dma_start(ag_sb[:, :], cc_out[:, :])
            else:
                # Hand-rolled all-gather, bypassing ncfw (~45us for an 8-byte
                # AllGather here): each core DMAs its [1,2] sums into row
                # <core_id> of a Shared DRAM buffer, signals all 8 peers via a
                # remote-sem broadcast (2 per dest), and reads the table back
                # once 16 signals arrived. Raw Pool-engine instructions with
                # nosync ordering edges -- each carries at most one sync wait,
                # which this walrus can encode (tile_critical cannot be used:
                # its entry branch wants one wait per live semaphore).
                nc.has_collectives = True  # maps the Shared scratchpad
                exch = nc.dram_tensor("exch", [8, 2], fp32, addr_space="Shared")
                g = nc.gpsimd
                pid = g.partition_id()
                s_w = nc.alloc_semaphore("exch_w")
                s_rem = nc.alloc_semaphore("exch_rem")
                _patch_sim_credit_remote_sem(s_rem)
                s_loc = nc.alloc_semaphore("exch_loc")
                i1 = g.dma_start(exch[bass.ds(pid, 1), 0:2], ssamp[0:1, 0:2])
                i1.then_inc(s_w, 16)
                i2 = g.wait_ge(s_w, 16)
                tile.add_dep_helper(i2.ins, i1.ins, sync=False, reason="exch w")
                i3 = g.remote_sem_update_broadcast(
                    remote_sem=s_rem, local_sem=s_loc,
                    rdests=[(0, k) for k in range(NCORES)],
                )
                tile.add_dep_helper(i3.ins, i2.ins, sync=False, reason="exch b")
                i4 = g.trigger_dma(count=None)
                tile.add_dep_helper(i4.ins, i3.ins, sync=False, reason="exch t")
                i5 = g.wait_ge(s_rem, 16)
                tile.add_dep_helper(i5.ins, i4.ins, sync=False, reason="exch p")
                i6 = g.dma_start(ag_sb[:, :], exch[0:8, 0:2])
                i6.then_inc(s_w, 16)
                tile.add_dep_helper(i6.ins, i5.ins, sync=False, reason="exch r")
                i7 = g.wait_ge(s_w, 32)
                tile.add_dep_helper(i7.ins, i6.ins, sync=False, reason="exch d")

            # broadcast the global sums to all partitions in the same matmul
            # that reduces the gathered rows: [8,128] ones^T @ [8,2]
            stotb_p = psum.tile([128, 2], fp32, name="stotb_p")
            nc.tensor.matmul(
                stotb_p[:, :], lhsT=ones_f[:, :], rhs=ag_sb[0:8, 0:2],
                start=True, stop=True,
            )
            # thb = max(stot*c, 0.01) broadcast; selection compares
            # exp(maxpool(ln x - ln u)) > th  (exp applied pre-collective)
            thb = small.tile([128, 2], fp32, name="thb")
            nc.vector.tensor_scalar(
                thb[:, 0:1], stotb_p[:, 0:1], 1.0 / (B * 500.0), 0.01, Alu.mult, Alu.max
            )
            nc.vector.tensor_scalar(
                thb[:, 1:2], stotb_p[:, 1:2], 1.0 / (B * 100.0), 0.01, Alu.mult, Alu.max
            )

            # ---- log-diff max-pools (DVE+GPSIMD) ---------------------------
            dt_s = big.tile([128, 2048], fp32, name="dt_s")
            nc.vector.tensor_sub(dt_s[:, :], lt[:, :], lut[:, :])
            pmt = small.tile([128, 128], fp32, name="pmt")
            nc.vector.tensor_reduce(
                out=pmt[:, :], in_=pool_view(dt_s[:, :]), axis=AX.XY, op=Alu.max
            )
            epmt = small.tile([128, 128], fp32, name="epmt")
            nc.scalar.activation(epmt[:, :], pmt[:, :], AF.Exp)
            dx_s = big.tile([128, 2048], fp32, name="dx_s")
            nc.vector.tensor_sub(dx_s[:, :], lx[:, :], lux[:, :])
            pmx = small.tile([128, 128], fp32, name="pmx")
            nc.vector.tensor_reduce(
                out=pmx[:, :], in_=pool_view(dx_s[:, :]), axis=AX.XY, op=Alu.max
            )
            epmx = small.tile([128, 128], fp32, name="epmx")
            nc.scalar.activation(epmx[:, :], pmx[:, :], AF.Exp)

            # ---- masked raw weights: q_raw = (pm > lth) * pooled ----------
            q_raw = small.tile([128, 128], fp32, name="q_raw")
            p_raw = small.tile([128, 128], fp32, name="p_raw")
            nc.vector.scalar_tensor_tensor(
                q_raw[:, :], epmx[:, :], thb[:, 0:1], xa[:, :], Alu.is_gt, Alu.mult
            )
            nc.vector.scalar_tensor_tensor(
                p_raw[:, :], epmt[:, :], thb[:, 1:2], ta[:, :], Alu.is_gt, Alu.mult
            )

            # ---- stats: [Sqq, Spp, Sqp, Zq, Zp] ----------------------------
            stats = small.tile([128, 8], fp32, name="stats")
            nc.vector.tensor_reduce(
                out=stats[:, 3:4], in_=q_raw[:, :], axis=AX.X, op=Alu.add
            )
            nc.vector.tensor_reduce(
                out=stats[:, 4:5], in_=p_raw[:, :], axis=AX.X, op=Alu.add
            )

            # Cq = K1 @ Qm @ K1 via two matmuls (K1 symmetric):
            #   Aq = matmul(lhsT=Qm, k1) = Qm^T K1 ; Cq = matmul(lhsT=Aq, k1)
            nc.tensor.matmul(aq_p[:, :], lhsT=q_raw[:, :], rhs=k1_s[:, :], start=True, stop=True)
            aq = small.tile([128, 128], fp32, name="aq")
            nc.scalar.copy(aq[:, :], aq_p[:, :])
            cq_p = psum.tile([128, 128], fp32, name="cq_p")
            nc.tensor.matmul(cq_p[:, :], lhsT=aq[:, :], rhs=k1_s[:, :], start=True, stop=True)

            ap_p = psum.tile([128, 128], fp32, name="ap_p")
            nc.tensor.matmul(ap_p[:, :], lhsT=p_raw[:, :], rhs=k1_s[:, :], start=True, stop=True)
            ap_s = small.tile([128, 128], fp32, name="ap_s")
            nc.scalar.copy(ap_s[:, :], ap_p[:, :])
            cp_p = psum.tile([128, 128], fp32, name="cp_p")
            nc.tensor.matmul(cp_p[:, :], lhsT=ap_s[:, :], rhs=k1_s[:, :], start=True, stop=True)

            junk0 = small.tile([128, 128], fp32, name="junk0")
            junk1 = small.tile([128, 128], fp32, name="junk1")
            junk2 = small.tile([128, 128], fp32, name="junk2")
            nc.vector.tensor_mul(junk0[:, :], q_raw[:, :], cq_p[:, :])
            nc.vector.tensor_reduce(
                out=stats[:, 0:1], in_=junk0[:, :], axis=AX.X, op=Alu.add
            )
            nc.vector.tensor_mul(junk1[:, :], p_raw[:, :], cp_p[:, :])
            nc.vector.tensor_reduce(
                out=stats[:, 1:2], in_=junk1[:, :], axis=AX.X, op=Alu.add
            )
            nc.vector.tensor_mul(junk2[:, :], q_raw[:, :], cp_p[:, :])
            nc.vector.tensor_reduce(
                out=stats[:, 2:3], in_=junk2[:, :], axis=AX.X, op=Alu.add
            )

            red_p = psum.tile([1, 8], fp32, name="red_p")
            nc.tensor.matmul(
                red_p[:, 0:5], lhsT=ones_p[:, :], rhs=stats[:, 0:5], start=True, stop=True
            )

            # ---- final scalar math (partition 0) ---------------------------
            invz = small.tile([1, 2], fp32, name="invz")
            nc.vector.reciprocal(invz[:, :], red_p[:, 3:5])
            v1 = small.tile([1, 2], fp32, name="v1")
            nc.vector.tensor_mul(v1[:, :], red_p[:, 0:2], invz[:, :])
            v2 = small.tile([1, 2], fp32, name="v2")
            nc.vector.tensor_mul(v2[:, :], v1[:, :], invz[:, :])
            s12 = small.tile([1, 1], fp32, name="s12")
            nc.vector.tensor_reduce(out=s12[:, :], in_=v2[:, :], axis=AX.X, op=Alu.add)
            ab = small.tile([1, 1], fp32, name="ab")
            nc.vector.tensor_mul(ab[:, :], invz[:, 0:1], invz[:, 1:2])
            t3 = small.tile([1, 1], fp32, name="t3")
            nc.vector.tensor_mul(t3[:, :], ab[:, :], red_p[:, 2:3])
            pos = small.tile([1, 1], fp32, name="pos")
            # pos = 0.5*s12 - t3
            nc.vector.scalar_tensor_tensor(
                pos[:, :], s12[:, :], 0.5, t3[:, :], Alu.mult, Alu.subtract
            )
            d = small.tile([1, 1], fp32, name="d")
            nc.vector.tensor_sub(d[:, :], ssamp[:, 0:1], ssamp[:, 1:2])
            d2 = small.tile([1, 1], fp32, name="d2")
            nc.vector.tensor_mul(d2[:, :], d[:, :], d[:, :])
            res_s = small.tile([1, 1], fp32, name="res_s")
            # res = d2/(256*262144) + pos
            nc.vector.scalar_tensor_tensor(
                res_s[:, :], d2[:, :], 1.0 / 67108864.0, pos[:, :], Alu.mult, Alu.add
            )
            if debug2:
                d2_d = nc.dram_tensor("dbg2", [1, 16], fp32, kind="ExternalOutput")
                d2t = small.tile([1, 16], fp32, name="d2t")
                nc.vector.memset(d2t[:, :], 0.0)
                nc.vector.tensor_copy(d2t[:, 0:1], res_s[:, :])
                nc.vector.tensor_copy(d2t[:, 1:3], ssamp[:, :])
                nc.vector.tensor_copy(d2t[:, 3:5], stotb_p[0:1, 0:2])
                nc.vector.tensor_copy(d2t[:, 5:7], thb[0:1, :])
                nc.vector.tensor_copy(d2t[:, 7:9], thb[0:1, :])
                nc.vector.tensor_copy(d2t[:, 9:14], red_p[:, 0:5])
                nc.gpsimd.dma_start(d2_d[:, :], d2t[:, :])

            nc.sync.dma_start(out_d[:, :], res_s[:, :])

            if debug:
                dbg_d = nc.dram_tensor("dbg", [128, 784], fp32, kind="ExternalOutput")
                dbg = big.tile([128, 784], fp32, name="dbg")
                nc.vector.memset(dbg[:, :], 0.0)
                nc.vector.tensor_copy(dbg[0:1, 0:2], ssamp[:, :])       # Sx, St
                nc.vector.tensor_copy(dbg[0:1, 2:4], stotb_p[0:1, 0:2])  # global sums
                nc.vector.tensor_copy(dbg[0:1, 4:6], thb[0:1, :])         # thresholds
                nc.vector.tensor_copy(dbg[0:1, 6:8], thb[0:1, :])         # thresholds2
                nc.vector.tensor_copy(dbg[0:1, 8:13], red_p[:, 0:5])    # Sqq Spp Sqp Zq Zp
                nc.vector.tensor_copy(dbg[0:1, 13:14], pos[:, :])
                nc.vector.tensor_copy(dbg[0:1, 14:15], d2[:, :])
                for k, tile_ in enumerate((xa, pmx, q_raw, ta, pmt, p_raw)):
                    nc.vector.tensor_copy(
                        dbg[:, 16 + 128 * k : 16 + 128 * (k + 1)], tile_[:, :]
                    )
                nc.gpsimd.dma_start(dbg_d[:, :], dbg[:, :])

    return nc


def _get_nc():
    if "nc" not in _CACHE:
        _CACHE["nc"] = _build_bass()
    return _CACHE["nc"]


def kernel(input, target, u_input, u_target):
    from concourse.bass_utils import run_bass_kernel_spmd

    nc = _get_nc()
    in_maps = []
    for b in range(NCORES):
        in_maps.append(
            {
                "x": np.ascontiguousarray(input[b].reshape(128, 2048), np.float32),
                "t": np.ascontiguousarray(target[b].reshape(128, 2048), np.float32),
                "ux": np.ascontiguousarray(u_input[b].reshape(128, 2048), np.float32),
                "ut": np.ascontiguousarray(u_target[b].reshape(128, 2048), np.float32),
            }
        )
    res = run_bass_kernel_spmd(nc, in_maps, core_ids=list(range(NCORES)))
    _CACHE["last_res"] = res
    out = np.array([res.results[b]["out"][0, 0] for b in range(NCORES)], np.float32)
    return out

